# revision 70
# baseline (speedup 1.0000x reference)
"""Trainium2 Bass kernel for nn_CausalCrossConditionalSelfAttention.

Strategy (8 NeuronCores, data-parallel over batch B=8, one element/core):
  - Host permutes tokens to interleaved temporal order => causal mask becomes
    lower-triangular, local mask a narrow band (+2 prefix cols).
  - All matmuls in bf16 (1 cyc/row at any N in the TRN2 cost model), psum fp32.
  - Scores computed transposed S^T[key, query] in width-bucketed blocks
    cropped to their true content span; fully-masked blocks skipped; partial
    blocks multiplied by host-precomputed 0/1 bf16 masks on DVE.
  - Conditional CLIP-token bias added via a rank-1 accumulate matmul
    (indicator-row x bias-row) on jb==0 score blocks only.
  - exp() batched: several score blocks share one 2-bank psum tile and one
    wide Act instruction (Act is the attention-phase co-bottleneck).
  - P@[V|z] gives unnormalized y plus softmax denominator as psum row 64;
    the per-head z column is pre-scaled by 1/mix so normalization is a plain
    4x-mode tensor_mul; Z -> DVE reciprocal (partition-0 row) -> gpsimd
    partition_broadcast -> per-softmax multiply into yTn bf16.
  - ml (mix) heads get duplicated v slots so their z scale can differ from
    the global heads sharing the same v.
  - Attention emitted as PAIRS of independent softmaxes interleaved
    unit-by-unit to fill the mm -> exp -> mask -> AV pipeline latency;
    q/k/v projections woven between pairs so PE stays dense (full clock).
  - Inputs land via few wide DMAs (per-DMA queue turnaround ~1.3us dominates
    many-small-transfer schedules); b_value/b_proj folded into a constant
    host-side output shift.

Self-contained: only needs numpy + ml_dtypes + the installed concourse stack.
"""
import sys

if "/opt/trn_rl_repo" not in sys.path:
    sys.path.insert(0, "/opt/trn_rl_repo")

import numpy as np
import ml_dtypes

# ----------------------------------------------------------------------------
# problem constants (hardcoded per spec)
# ----------------------------------------------------------------------------
BLOCK = 512
RECEP = 4
N_HEAD = 8
EMBED = 512
HS = 64
T = 2 * BLOCK + 2          # 1026
TP = 1152                  # 9 * 128
NJB = TP // 128
NIC = 3
AW = (384, 384, 258)       # attention query-chunk widths (sum = T = 1026)
IOFF = (0, 384, 768)       # chunk start columns
NSM = 10
NCORES = 8
BF = ml_dtypes.bfloat16

# softmax id -> (mask kind, q/k source, v head)
SM_INFO = [
    (0, "loc", "main", 0), (1, "loc", "main", 1),
    (2, "seq", "main", 2), (3, "seq", "main", 3),
    (4, "seq", "main", 4), (5, "seq", "main", 5),
    (6, "seq", "main", 6), (7, "seq", "main", 7),
    (8, "loc", "ml", 8), (9, "loc", "ml", 9),
]


# ----------------------------------------------------------------------------
# host-side plan construction
# ----------------------------------------------------------------------------
def build_perm():
    perm = np.zeros(T, dtype=np.int64)
    perm[0], perm[1] = 0, 1
    b = np.arange(BLOCK)
    perm[2 + 2 * b] = 2 + b
    perm[3 + 2 * b] = 2 + BLOCK + b
    inv = np.argsort(perm)
    return perm, inv


def build_masks_orig():
    to = np.concatenate([np.zeros(2), np.arange(BLOCK) * 2 + 1, np.arange(BLOCK) * 2 + 2])
    seq = to[None, :] <= to[:, None]
    qo = np.concatenate([np.arange(BLOCK) * 2 + 1 - 2 * RECEP + 1] * 2)
    ko = np.concatenate([np.arange(BLOCK) * 2 + 1] * 2)
    de = ko[None, :] < qo[:, None]
    loc = seq.copy()
    loc[2:, 2:] = loc[2:, 2:] & (~de)
    return seq, loc


def build_units():
    """Per (kind, ic): list of units.

    unit = dict(rows, w, nfit, blocks=[dict(jb, a0, mid)]); blocks in a unit
    share (rows, w); psum layout: single-bank packed (nfit>1, chunk k at
    col k*w) or one bank per block (nfit==1, chunk k at col k*512).
    Unit 0 contains jb==0 (AV start flag / CLIP bias matmul target).
    """
    perm, _ = build_perm()
    seq, loc = build_masks_orig()
    mask_tiles, tile_index = [], {}

    def tile_id(slab):
        key = (slab.shape[1], slab.tobytes())
        if key not in tile_index:
            tile_index[key] = len(mask_tiles)
            mask_tiles.append(slab.astype(np.float32))
        return tile_index[key]

    plans = {}
    for kind, M0 in (("seq", seq), ("loc", loc)):
        Mp = np.zeros((TP, TP), dtype=bool)
        Mp[:T, :T] = M0[perm][:, perm]
        icunits = []
        for ic in range(NIC):
            i0, W = IOFF[ic], AW[ic]
            blocks = []
            for jb in range(NJB):
                sub = Mp[i0:i0 + W, jb * 128:(jb + 1) * 128].T.copy()  # [128 keys, W]
                if not sub.any():
                    continue
                nzr = np.flatnonzero(sub.any(axis=1))
                rows = 2 if nzr.max() <= 1 else 128
                nzc = np.flatnonzero(sub.any(axis=0))
                a0 = int(nzc.min()) & ~1
                a1 = min(W, (int(nzc.max()) + 2) & ~1)
                blocks.append((jb, rows, a0, a1, sub))
            # classes by (rows, 128-col width bucket); unify width per class
            cls = {}
            for b in blocks:
                jb, rows, a0, a1, sub = b
                cls.setdefault((rows, -(-(a1 - a0) // 128)), []).append(b)
            units = []
            for (rows, _wb), bl in sorted(cls.items(), key=lambda kv: min(b[0] for b in kv[1])):
                w = min(W, max(b[3] - b[2] for b in bl))
                nfit = (512 // w) if w < 256 else 1
                cap = nfit if nfit > 1 else 2
                cur = []
                for jb, brows, a0, a1, sub in bl:
                    a0 = max(0, min(a0, W - w)) & ~1
                    slab = sub[:, a0:a0 + w]
                    if slab[0:rows].all():
                        mid, m0 = None, 0
                    else:
                        # multiply only columns that are not all-ones (this
                        # includes every all-zero column, so crop is safe)
                        nfull = np.flatnonzero(~slab[0:rows].all(axis=0))
                        m0 = int(nfull.min()) & ~1
                        m1 = min(w, (int(nfull.max()) + 2) & ~1)
                        mid = tile_id(slab[:, m0:m1])
                    cur.append(dict(jb=jb, a0=a0, mid=mid, m0=m0))
                    if len(cur) == cap:
                        units.append(dict(rows=rows, w=w, nfit=nfit, blocks=cur))
                        cur = []
                if cur:
                    units.append(dict(rows=rows, w=w, nfit=nfit, blocks=cur))
            units.sort(key=lambda u: min(b["jb"] for b in u["blocks"]))
            assert units[0]["blocks"][0]["jb"] == 0
            icunits.append(units)
        plans[kind] = icunits

    offs, cat, o = [], [], 0
    for t in mask_tiles:
        offs.append((o, t.shape[1]))
        cat.append(t)
        o += t.shape[1]
    maskcat = np.concatenate(cat, axis=1) if cat else np.zeros((128, 2), np.float32)
    return plans, maskcat, offs


def host_const_shift(w):
    bv = w["b_value"].astype(np.float64)
    wg = w["w_mix"].astype(np.float64)[:, 0, 0, 0]
    wl = w["w_mix"].astype(np.float64)[:, 1, 0, 0]
    scale_h = np.ones(N_HEAD)
    scale_h[2] = wg[0] + wl[0]
    scale_h[3] = wg[1] + wl[1]
    yshift = (bv.reshape(N_HEAD, HS) * scale_h[:, None]).reshape(-1)
    return (yshift @ w["w_proj"].astype(np.float64).T
            + w["b_proj"].astype(np.float64)).astype(np.float32)


def prep_shared(w):
    """Shared (weight) tensors, bf16 where they feed matmuls."""
    f = np.float32
    scale = np.float32(1.0 / np.sqrt(HS))
    out = {}
    out["wq"] = np.ascontiguousarray(w["w_query"].astype(f).T * scale).astype(BF)
    out["wk"] = np.ascontiguousarray(w["w_key"].astype(f).T).astype(BF)
    out["wv"] = np.ascontiguousarray(w["w_value"].astype(f).T).astype(BF)
    out["wp"] = np.ascontiguousarray(w["w_proj"].astype(f).T).astype(BF)
    out["wqml"] = np.ascontiguousarray(w["w_query_ml"].astype(f).T * scale).astype(BF)
    out["wkml"] = np.ascontiguousarray(w["w_key_ml"].astype(f).T).astype(BF)
    # merged per-partition constants: [bq(4) | bk(4) | bqml | bkml | mixbc(10)]
    consts = np.zeros((128, 20), dtype=f)
    consts[:, 0:4] = (w["b_query"].astype(f) * scale).reshape(4, 128).T
    consts[:, 4:8] = w["b_key"].astype(f).reshape(4, 128).T
    consts[:, 8] = w["b_query_ml"].astype(f) * scale
    consts[:, 9] = w["b_key_ml"].astype(f)
    wg = w["w_mix"].astype(f)[:, 0, 0, 0]
    wl = w["w_mix"].astype(f)[:, 1, 0, 0]
    mix = np.ones(NSM, dtype=f)
    mix[2], mix[3] = wg[0], wg[1]
    mix[8], mix[9] = wl[0], wl[1]
    consts[:, 10:20] = mix[None, :]
    out["consts"] = consts
    mixc = np.where(np.abs(mix) < 1e-6, 1e-6, mix)
    out["zcols"] = np.tile((1.0 / mixc)[None, :], (128, 1)).astype(BF)

    ind = np.zeros((1, TP), dtype=f)
    ind[0, 1] = 1.0
    out["indrow"] = ind.astype(BF)
    return out


def prep_biasrow(w, cond_b):
    f = np.float32
    br = np.zeros((1, NSM * TP), dtype=f)
    if cond_b > 0:
        clip8 = np.maximum(w["att_bias_clip"].astype(f)[0, :, 0], 0.0) * 10.0
        clip2 = np.maximum(w["att_bias_clip_ml"].astype(f)[0, :, 0], 0.0) * 10.0
        for s in range(N_HEAD):
            br[0, s * TP:(s + 1) * TP] = clip8[s]
        for j in range(2):
            br[0, (N_HEAD + j) * TP:(N_HEAD + j + 1) * TP] = clip2[j]
    return br.astype(BF)


def prep_xT(x_b, perm):
    xT = np.zeros((EMBED, TP), dtype=np.float32)
    xT[:, :T] = x_b[perm].T
    return xT.astype(BF)


# ----------------------------------------------------------------------------
# bass kernel emission
# ----------------------------------------------------------------------------
def emit_kernel(tc, ins, out_ap, plans, mask_offs):
    from contextlib import ExitStack
    from concourse import mybir

    nc = tc.nc
    f32 = mybir.dt.float32
    bf16 = mybir.dt.bfloat16
    AF = mybir.ActivationFunctionType
    MULT = mybir.AluOpType.mult

    with ExitStack() as ctx:
        P = ctx.enter_context(tc.tile_pool(name="persist", bufs=1))

        # ---------------- persistent tiles ----------------
        # x and weights live in single wide tiles (kc chunks along the free
        # dim) so each loads with one or two big DMAs — per-DMA queue
        # turnaround (~1.3us) dominates many-small-transfer schedules.
        xTb = P.tile([128, 4 * TP], bf16, name="xTb", tag="xTb")
        qT = [P.tile([128, TP], bf16, name=f"qT{m}", tag=f"qT{m}") for m in range(4)]
        kT = [P.tile([128, TP], bf16, name=f"kT{m}", tag=f"kT{m}") for m in range(4)]
        qml = P.tile([128, TP], bf16, name="qml", tag="qml")
        kml = P.tile([128, TP], bf16, name="kml", tag="kml")
        vext = [P.tile([128, 650], bf16, name=f"vext{t}", tag=f"vext{t}")
                for t in range(NJB)]
        ytmp = P.tile([65, NSM * TP], bf16, name="ytmp", tag="ytmp")
        yt_z = P.tile([1, NSM * TP], bf16, name="yt_z", tag="yt_z")
        yTn = [P.tile([128, TP], bf16, name=f"yTn{p}", tag=f"yTn{p}") for p in range(4)]
        tmpml = P.tile([128, TP], bf16, name="tmpml", tag="tmpml")

        def xs(kc, c0, c1):
            return xTb[:, kc * TP + c0:kc * TP + c1]

        def loadw(name, nkc, ncols, eng):
            """Whole [512, ncols] weight as one [128, 4*ncols] tile, one DMA."""
            t = P.tile([128, nkc * ncols], bf16, name=name, tag=name)
            eng.dma_start(t[:].rearrange("p (g c) -> p g c", c=ncols),
                          ins[name].rearrange("(g p) c -> p g c", p=128))
            return t

        # ---------------- input loads ----------------
        # first projection group needs x(ic0) + wq only; interleave queues.
        wqb = loadw("wq", 4, 512, nc.sync)
        wkb = loadw("wk", 4, 512, nc.scalar)
        for ic in range(NIC):
            c0, c1 = ic * 384, (ic + 1) * 384
            nc.sync.dma_start(
                xTb[:].rearrange("p (g c) -> p g c", c=TP)[:, 0:2, c0:c1],
                ins["xT"].rearrange("(g p) c -> p g c", p=128)[:, 0:2, c0:c1])
            nc.scalar.dma_start(
                xTb[:].rearrange("p (g c) -> p g c", c=TP)[:, 2:4, c0:c1],
                ins["xT"].rearrange("(g p) c -> p g c", p=128)[:, 2:4, c0:c1])
        wvb = loadw("wv", 4, 512, nc.sync)
        wqmlb = loadw("wqml", 4, 128, nc.scalar)
        wkmlb = loadw("wkml", 4, 128, nc.scalar)
        consts_sb = P.tile([128, 20], f32, name="consts", tag="consts")
        nc.scalar.dma_start(consts_sb[:], ins["consts"][:, :])
        zcols_sb = P.tile([128, NSM], bf16, name="zcols", tag="zcols")
        nc.scalar.dma_start(zcols_sb[:], ins["zcols"][:, :])
        indrow_sb = P.tile([1, TP], bf16, name="indrow", tag="indrow")
        nc.scalar.dma_start(indrow_sb[:], ins["indrow"][:, :])
        biasrow_sb = P.tile([1, NSM * TP], bf16, name="biasrow", tag="biasrow")
        nc.sync.dma_start(biasrow_sb[:], ins["biasrow"][:, :])
        maskw = ins["masks"].shape[1]
        maskcat_sb = P.tile([128, maskw], bf16, name="masks", tag="masks")
        nc.sync.dma_start(maskcat_sb[:], ins["masks"][:, :])

        def proj_group(wtile, bcol, m, dst, on_act=False):
            """One output m-chunk of a projection: 2 psum tiles + 2 evacs."""
            ps1 = _SP3[0].tile([128, 1024], f32, name="pp", tag="sp")
            ps2 = _SP3[0].tile([128, 1024], f32, name="pp2", tag="sp")
            for ic in range(NIC):
                pw = AW[ic]
                ps, o = (ps1, ic * 512) if ic < 2 else (ps2, 0)
                for kc in range(4):
                    nc.tensor.matmul(
                        ps[:, o:o + pw],
                        lhsT=wtile[:, kc * (wtile.shape[1] // 4) + m * 128:
                                   kc * (wtile.shape[1] // 4) + (m + 1) * 128],
                        rhs=xs(kc, ic * 384, ic * 384 + pw),
                        start=(kc == 0), stop=(kc == 3))
            bias = consts_sb[:, bcol:bcol + 1]
            dv1 = dst[:, 0:768].rearrange("p (g w) -> p g w", w=384)
            pv1 = ps1[:].rearrange("p (g c) -> p g c", c=512)[:, :, 0:384]
            # only cols 768:1026 are ever read downstream (jb8 blocks crop
            # to 2 key rows); do not read unwritten psum (stale on hardware)
            if on_act:
                nc.scalar.activation(dv1, pv1, AF.Identity, bias=bias)
                nc.scalar.activation(dst[:, 768:1026], ps2[:, 0:258],
                                     AF.Identity, bias=bias)
            else:
                nc.vector.tensor_scalar_add(dv1, pv1, bias)
                nc.vector.tensor_scalar_add(dst[:, 768:1026], ps2[:, 0:258], bias)

        def att_ic_units(s, ic):
            _, kindname, src_, hv = SM_INFO[s]
            if src_ == "main":
                qt, kt, off = qT[s // 2], kT[s // 2], (s % 2) * 64
                assert s // 2 in emitted_qk, (s, ic)
            else:
                qt, kt, off = qml, kml, (s - N_HEAD) * 64
                assert "ml" in emitted_qk, (s, ic)
            i0, W = IOFF[ic], AW[ic]
            units = plans[kindname][ic]
            n_av = sum(len(u["blocks"]) for u in units)
            Y = _YP[0].tile([128, 512], f32, name="y", tag="y")
            avi = 0
            for u in units:
                rows, w, nfit, blocks = u["rows"], u["w"], u["nfit"], u["blocks"]
                nb = len(blocks)
                ps = _SP3[0].tile([128, 1024], f32, name="sp", tag="sp")
                poffs = [(k // nfit) * 512 + (k % nfit) * w for k in range(nb)]
                for k, b in enumerate(blocks):
                    jb, a0 = b["jb"], b["a0"]
                    o = poffs[k]
                    first = (k % nfit == 0)
                    last = (k % nfit == nfit - 1) or (k == nb - 1)
                    nc.tensor.matmul(
                        ps[0:rows, o:o + w],
                        lhsT=kt[off:off + 64, jb * 128:jb * 128 + rows],
                        rhs=qt[off:off + 64, i0 + a0:i0 + a0 + w],
                        start=first, stop=last and (jb != 0))
                    if jb == 0:
                        nc.tensor.matmul(
                            ps[0:rows, o:o + w],
                            lhsT=indrow_sb[0:1, 0:rows],
                            rhs=biasrow_sb[0:1, s * TP + i0 + a0:s * TP + i0 + a0 + w],
                            start=False, stop=last)
                pt = _PTP[0].tile([128, 1536], bf16, name="pt", tag="pt")
                if nfit == 1:
                    pin = ps[0:rows, :].rearrange("p (g c) -> p g c", c=512)[:, 0:nb, 0:w]
                    pout = pt[0:rows, 0:nb * w].rearrange("p (g c) -> p g c", c=w)
                else:
                    pin = ps[0:rows, 0:nb * w]
                    pout = pt[0:rows, 0:nb * w]
                nc.scalar.activation(pout, pin, AF.Exp)
                for k, b in enumerate(blocks):
                    if b["mid"] is not None:
                        mo, mw = mask_offs[b["mid"]]
                        o0 = k * w + b["m0"]
                        nc.vector.tensor_mul(pt[0:rows, o0:o0 + mw],
                                             pt[0:rows, o0:o0 + mw],
                                             maskcat_sb[0:rows, mo:mo + mw])
                for k, b in enumerate(blocks):
                    a0 = b["a0"]
                    assert b["jb"] in emitted_v, (s, ic, b["jb"])
                    nc.tensor.matmul(
                        Y[0:65, a0:a0 + w],
                        lhsT=vext[b["jb"]][0:rows, hv * 65:hv * 65 + 65],
                        rhs=pt[0:rows, k * w:(k + 1) * w],
                        start=(avi == 0), stop=(avi == n_av - 1))
                    avi += 1
                yield
            nc.vector.tensor_copy(ytmp[0:65, s * TP + i0:s * TP + i0 + W],
                                  Y[0:65, 0:W])
            norm_z(s, ic)

        rb_live = {}

        def norm_z(s, ic):
            # Z for (s, ic) is complete with that chunk (it sums over keys):
            # 1/Z from ytmp row 64 to a partition-0 row (DVE 64->0 shift),
            # then broadcast immediately; the multiply is deferred.
            i0, W = IOFF[ic], AW[ic]
            c0 = s * TP + i0
            with nc.allow_low_precision(reason="softmax normalizer"):
                nc.vector.reciprocal(yt_z[0:1, c0:c0 + W],
                                     ytmp[64:65, c0:c0 + W])
            rb = _RBP[0].tile([128, 384], bf16, name="zb", tag="zb")
            nc.gpsimd.partition_broadcast(rb[0:128, 0:W],
                                          yt_z[0:1, c0:c0 + W], channels=128)
            rb_live[(s, ic)] = rb

        def norm(s, ic):
            i0, W = IOFF[ic], AW[ic]
            c0 = s * TP + i0
            rb = rb_live.pop((s, ic))
            if s < N_HEAD:
                dst = yTn[s // 2][(s % 2) * 64:(s % 2) * 64 + 64, i0:i0 + W]
            else:
                dst = tmpml[(s - N_HEAD) * 64:(s - N_HEAD) * 64 + 64, i0:i0 + W]
            nc.vector.tensor_mul(dst, ytmp[0:64, c0:c0 + W], rb[0:64, 0:W])

        emitted_v = set()
        emitted_qk = {1, "ml0"}

        def v_group(tt, pool, on_act, tag="vp"):
            emitted_v.add(tt)
            ps = pool.tile([128, 1024], f32, name="vp", tag=tag)
            for kc in range(4):
                nc.tensor.matmul(
                    ps[:, 0:512],
                    lhsT=xs(kc, tt * 128, (tt + 1) * 128),
                    rhs=wvb[:, kc * 512:(kc + 1) * 512],
                    start=(kc == 0), stop=(kc == 3))
            vx = vext[tt][:].rearrange("p (h e) -> p h e", e=65)
            pv = ps[:, 0:512].rearrange("p (h d) -> p h d", d=64)
            if on_act:
                nc.scalar.activation(vx[:, 0:8, 0:64], pv, AF.Copy)
            else:
                nc.vector.tensor_copy(vx[:, 0:8, 0:64], pv)
            # ml softmaxes use duplicated v slots (8, 9) for heads 2, 3 so
            # their Z column can carry a different 1/mix scale
            nc.vector.tensor_copy(vx[:, 8:10, 0:64],
                                  ps[:, 128:256].rearrange("p (h d) -> p h d", d=64))
            nc.gpsimd.tensor_copy(vx[:, :, 64:65], zcols_sb[:, :, None])

        # ---------------- phase A: q1/k1 + v projections ----------------
        with tc.tile_pool(name="pps", bufs=2, space="PSUM") as pps, \
             tc.tile_pool(name="vps", bufs=2, space="PSUM") as vps:
            _SP3 = [pps]
            proj_group(wqb, 0 + 1, 1, qT[1])
            proj_group(wkb, 4 + 1, 1, kT[1])
            for tt in range(3):
                v_group(tt, vps, True)

        wpb = loadw("wp", 4, 512, nc.sync)

        # ---------------- phase B: interleaved projections + attention ----
        with tc.tile_pool(name="sp3", bufs=3, space="PSUM") as sp3, \
             tc.tile_pool(name="yp", bufs=2, space="PSUM") as yp, \
             tc.tile_pool(name="ptp", bufs=8) as ptp, \
             tc.tile_pool(name="rbp", bufs=4) as rbp:
            _SP3[0] = sp3
            _YP = [yp]
            _PTP = [ptp]
            _RBP = [rbp]

            # zero the padding columns of the normalized tiles once
            for tile_ in yTn + [tmpml]:
                nc.gpsimd.memset(tile_[:, T:TP], 0.0)

            pending = []
            normed = set()
            added = [False]
            need_add = {(s_, ic_) for s_ in (2, 3, 8, 9) for ic_ in range(NIC)}

            def norm_and_track(key):
                norm(*key)
                normed.add(key)
                if not added[0] and need_add <= normed:
                    nc.vector.tensor_add(yTn[1][:, 0:T], yTn[1][:, 0:T],
                                         tmpml[:, 0:T])
                    added[0] = True

            def pq(m):
                def f():
                    proj_group(wqb, 0 + m, m, qT[m])
                return f

            def pk(m):
                def f():
                    proj_group(wkb, 4 + m, m, kT[m])
                    emitted_qk.add(m)
                return f

            def pmlq():
                proj_group(wqmlb, 8, 0, qml)

            def pmlk():
                proj_group(wkmlb, 9, 0, kml)
                emitted_qk.add("ml")

            bgp = []
            bgv = [[lambda tt=tt: v_group(tt, sp3, True, "sp")
                    for tt in (3, 4, 5)],
                   [lambda tt=tt: v_group(tt, sp3, True, "sp")
                    for tt in (6, 7, 8)]]

            def A2(sa, sb):
                # two independent softmaxes interleaved unit-by-unit: each
                # engine always has work from the other chain to fill the
                # mm -> exp -> mask -> AV pipeline latency. One background
                # projection group and the deferred norm multiplies are
                # emitted at each chunk boundary.
                for ic in range(NIC):
                    ga = att_ic_units(sa, ic)
                    gb = att_ic_units(sb, ic)
                    alive = [ga, gb]
                    while alive:
                        for g in list(alive):
                            try:
                                next(g)
                            except StopIteration:
                                alive.remove(g)
                    if bgv and ic < 2:
                        for f in bgv.pop(0):
                            f()
                    for _ in range(2):
                        if pending:
                            norm_and_track(pending.pop(0))
                    pending.extend([(sa, ic), (sb, ic)])

            pair_work = [(2, 3), (4, 0), (5, 8), (6, 1), (7, 9)]
            sched = [pq(2), pk(2), None, pq(0), pk(0), None, pmlq, pmlk, None,
                     pq(3), pk(3), None, None]
            ai = 0
            for item in sched:
                if item is None:
                    A2(*pair_work[ai])
                    ai += 1
                else:
                    item()
            for s_ in pending:
                norm_and_track(s_)

        # ---------------- phase C: output projection ----------------
        with tc.tile_pool(name="ops", bufs=2, space="PSUM") as ops, \
             tc.tile_pool(name="ostage", bufs=3) as ostage:
            ost = None
            for m in range(NJB):
                po = ops.tile([128, 512], f32, name="po", tag="po")
                for i, p in enumerate((2, 0, 3, 1)):
                    nc.tensor.matmul(
                        po[:],
                        lhsT=yTn[p][:, m * 128:(m + 1) * 128],
                        rhs=wpb[:, p * 512:(p + 1) * 512],
                        start=(i == 0), stop=(i == 3))
                if m % 2 == 0:
                    ost = ostage.tile([128, 1024], f32, name="ost", tag="ost")
                    nc.scalar.activation(ost[:, 0:512], po[:], AF.Copy)
                    if m == NJB - 1:
                        nc.sync.dma_start(out_ap[m * 128:(m + 1) * 128, :],
                                          ost[:, 0:512])
                else:
                    nc.vector.tensor_copy(ost[:, 512:1024], po[:])
                    eng = nc.sync if m % 4 == 1 else nc.scalar
                    eng.dma_start(
                        out_ap[(m - 1) * 128:(m + 1) * 128, :].rearrange(
                            "(g p) c -> p g c", p=128),
                        ost[:].rearrange("p (g c) -> p g c", c=512))


# ----------------------------------------------------------------------------
# module build + run
# ----------------------------------------------------------------------------
_CACHE = {}


def _get_module():
    if "nc" in _CACHE:
        return _CACHE["nc"], _CACHE["maskcat"]
    import concourse.tile as tile
    from concourse import bacc, mybir

    plans, maskcat, mask_offs = build_units()

    nc = bacc.Bacc("TRN2", target_bir_lowering=False, debug=False,
                   enable_asserts=False, num_devices=NCORES)
    f32 = mybir.dt.float32
    bf16 = mybir.dt.bfloat16

    def din(name, shape, dt=f32):
        return nc.dram_tensor(name, list(shape), dt, kind="ExternalInput").ap()

    ins = dict(
        xT=din("xT", (EMBED, TP), bf16),
        wq=din("wq", (EMBED, EMBED), bf16), wk=din("wk", (EMBED, EMBED), bf16),
        wv=din("wv", (EMBED, EMBED), bf16), wp=din("wp", (EMBED, EMBED), bf16),
        wqml=din("wqml", (EMBED, 128), bf16), wkml=din("wkml", (EMBED, 128), bf16),
        consts=din("consts", (128, 20)),
        zcols=din("zcols", (128, NSM), bf16),
        indrow=din("indrow", (1, TP), bf16),
        biasrow=din("biasrow", (1, NSM * TP), bf16),
        masks=din("masks", (128, maskcat.shape[1]), bf16),
    )
    out_ap = nc.dram_tensor("out_p", [TP, EMBED], f32, kind="ExternalOutput").ap()

    with tile.TileContext(nc) as tc:
        emit_kernel(tc, ins, out_ap, plans, mask_offs)
    nc.compile()

    _CACHE.update(nc=nc, maskcat=maskcat.astype(BF))
    return nc, _CACHE["maskcat"]


def build_in_maps(inputs):
    nc, maskcat = _get_module()
    x = inputs["x"].astype(np.float32)
    cond = np.asarray(inputs["cond_mask"]).astype(np.int32)
    B = x.shape[0]
    assert B == NCORES, f"expected B={NCORES}, got {B}"

    perm, _ = build_perm()
    shared = prep_shared(inputs)
    shared["masks"] = maskcat
    br_cache = {}
    in_maps = []
    for b in range(B):
        ci = dict(shared)
        ci["xT"] = prep_xT(x[b], perm)
        cb = int(cond[b])
        if cb not in br_cache:
            br_cache[cb] = prep_biasrow(inputs, cb)
        ci["biasrow"] = br_cache[cb]
        in_maps.append(ci)
    return nc, in_maps


def kernel(**inputs):
    from concourse import bass_utils

    inputs = {k: np.asarray(v) for k, v in inputs.items()}
    nc, in_maps = build_in_maps(inputs)
    res = bass_utils.run_bass_kernel_spmd(nc, in_maps, core_ids=list(range(NCORES)))
    _CACHE["last_results"] = res

    _, inv = build_perm()
    shift = host_const_shift(inputs)
    B = inputs["x"].shape[0]
    out = np.empty((B, T, EMBED), dtype=np.float32)
    for b in range(B):
        out[b] = res.results[b]["out_p"][:T][inv] + shift
    return out


# revision 71
# speedup vs baseline: 1.0167x; 1.0167x over previous
"""Trainium2 Bass kernel for nn_CausalCrossConditionalSelfAttention.

Strategy (8 NeuronCores, data-parallel over batch B=8, one element/core):
  - Host permutes tokens to interleaved temporal order => causal mask becomes
    lower-triangular, local mask a narrow band (+2 prefix cols).
  - All matmuls in bf16 (1 cyc/row at any N in the TRN2 cost model), psum fp32.
  - Scores computed transposed S^T[key, query] in width-bucketed blocks
    cropped to their true content span; fully-masked blocks skipped; partial
    blocks multiplied by host-precomputed 0/1 bf16 masks on DVE.
  - Conditional CLIP-token bias added via a rank-1 accumulate matmul
    (indicator-row x bias-row) on jb==0 score blocks only.
  - exp() batched: several score blocks share one 2-bank psum tile and one
    wide Act instruction (Act is the attention-phase co-bottleneck).
  - P@[V|z] gives unnormalized y plus softmax denominator as psum row 64;
    the per-head z column is pre-scaled by 1/mix so normalization is a plain
    4x-mode tensor_mul; Z -> DVE reciprocal (partition-0 row) -> gpsimd
    partition_broadcast -> per-softmax multiply into yTn bf16.
  - ml (mix) heads get duplicated v slots so their z scale can differ from
    the global heads sharing the same v.
  - Attention emitted as PAIRS of independent softmaxes interleaved
    unit-by-unit to fill the mm -> exp -> mask -> AV pipeline latency;
    q/k/v projections woven between pairs so PE stays dense (full clock).
  - Inputs land via few wide DMAs (per-DMA queue turnaround ~1.3us dominates
    many-small-transfer schedules); b_value/b_proj folded into a constant
    host-side output shift.

Self-contained: only needs numpy + ml_dtypes + the installed concourse stack.
"""
import sys

if "/opt/trn_rl_repo" not in sys.path:
    sys.path.insert(0, "/opt/trn_rl_repo")

import numpy as np
import ml_dtypes

# ----------------------------------------------------------------------------
# problem constants (hardcoded per spec)
# ----------------------------------------------------------------------------
BLOCK = 512
RECEP = 4
N_HEAD = 8
EMBED = 512
HS = 64
T = 2 * BLOCK + 2          # 1026
TP = 1152                  # 9 * 128
NJB = TP // 128
NIC = 3
AW = (384, 384, 258)       # attention query-chunk widths (sum = T = 1026)
IOFF = (0, 384, 768)       # chunk start columns
NSM = 10
NCORES = 8
BF = ml_dtypes.bfloat16

# softmax id -> (mask kind, q/k source, v head)
SM_INFO = [
    (0, "loc", "main", 0), (1, "loc", "main", 1),
    (2, "seq", "main", 2), (3, "seq", "main", 3),
    (4, "seq", "main", 4), (5, "seq", "main", 5),
    (6, "seq", "main", 6), (7, "seq", "main", 7),
    (8, "loc", "ml", 8), (9, "loc", "ml", 9),
]


# ----------------------------------------------------------------------------
# host-side plan construction
# ----------------------------------------------------------------------------
def build_perm():
    perm = np.zeros(T, dtype=np.int64)
    perm[0], perm[1] = 0, 1
    b = np.arange(BLOCK)
    perm[2 + 2 * b] = 2 + b
    perm[3 + 2 * b] = 2 + BLOCK + b
    inv = np.argsort(perm)
    return perm, inv


def build_masks_orig():
    to = np.concatenate([np.zeros(2), np.arange(BLOCK) * 2 + 1, np.arange(BLOCK) * 2 + 2])
    seq = to[None, :] <= to[:, None]
    qo = np.concatenate([np.arange(BLOCK) * 2 + 1 - 2 * RECEP + 1] * 2)
    ko = np.concatenate([np.arange(BLOCK) * 2 + 1] * 2)
    de = ko[None, :] < qo[:, None]
    loc = seq.copy()
    loc[2:, 2:] = loc[2:, 2:] & (~de)
    return seq, loc


def build_units():
    """Per (kind, ic): list of units.

    unit = dict(rows, w, nfit, blocks=[dict(jb, a0, mid)]); blocks in a unit
    share (rows, w); psum layout: single-bank packed (nfit>1, chunk k at
    col k*w) or one bank per block (nfit==1, chunk k at col k*512).
    Unit 0 contains jb==0 (AV start flag / CLIP bias matmul target).
    """
    perm, _ = build_perm()
    seq, loc = build_masks_orig()
    mask_tiles, tile_index = [], {}

    def tile_id(slab):
        key = (slab.shape[1], slab.tobytes())
        if key not in tile_index:
            tile_index[key] = len(mask_tiles)
            mask_tiles.append(slab.astype(np.float32))
        return tile_index[key]

    plans = {}
    for kind, M0 in (("seq", seq), ("loc", loc)):
        Mp = np.zeros((TP, TP), dtype=bool)
        Mp[:T, :T] = M0[perm][:, perm]
        icunits = []
        for ic in range(NIC):
            i0, W = IOFF[ic], AW[ic]
            blocks = []
            for jb in range(NJB):
                sub = Mp[i0:i0 + W, jb * 128:(jb + 1) * 128].T.copy()  # [128 keys, W]
                if not sub.any():
                    continue
                nzr = np.flatnonzero(sub.any(axis=1))
                rows = 2 if nzr.max() <= 1 else 128
                nzc = np.flatnonzero(sub.any(axis=0))
                a0 = int(nzc.min()) & ~1
                a1 = min(W, (int(nzc.max()) + 2) & ~1)
                blocks.append((jb, rows, a0, a1, sub))
            # classes by (rows, 128-col width bucket); unify width per class
            cls = {}
            for b in blocks:
                jb, rows, a0, a1, sub = b
                cls.setdefault((rows, -(-(a1 - a0) // 128)), []).append(b)
            units = []
            for (rows, _wb), bl in sorted(cls.items(), key=lambda kv: min(b[0] for b in kv[1])):
                w = min(W, max(b[3] - b[2] for b in bl))
                nfit = (512 // w) if w < 256 else 1
                cap = nfit if nfit > 1 else 2
                cur = []
                for jb, brows, a0, a1, sub in bl:
                    a0 = max(0, min(a0, W - w)) & ~1
                    slab = sub[:, a0:a0 + w]
                    if slab[0:rows].all():
                        mid, m0 = None, 0
                    else:
                        # multiply only columns that are not all-ones (this
                        # includes every all-zero column, so crop is safe)
                        nfull = np.flatnonzero(~slab[0:rows].all(axis=0))
                        m0 = int(nfull.min()) & ~1
                        m1 = min(w, (int(nfull.max()) + 2) & ~1)
                        mid = tile_id(slab[:, m0:m1])
                    cur.append(dict(jb=jb, a0=a0, mid=mid, m0=m0))
                    if len(cur) == cap:
                        units.append(dict(rows=rows, w=w, nfit=nfit, blocks=cur))
                        cur = []
                if cur:
                    units.append(dict(rows=rows, w=w, nfit=nfit, blocks=cur))
            units.sort(key=lambda u: min(b["jb"] for b in u["blocks"]))
            assert units[0]["blocks"][0]["jb"] == 0
            icunits.append(units)
        plans[kind] = icunits

    offs, cat, o = [], [], 0
    for t in mask_tiles:
        offs.append((o, t.shape[1]))
        cat.append(t)
        o += t.shape[1]
    maskcat = np.concatenate(cat, axis=1) if cat else np.zeros((128, 2), np.float32)
    return plans, maskcat, offs


def host_const_shift(w):
    bv = w["b_value"].astype(np.float64)
    wg = w["w_mix"].astype(np.float64)[:, 0, 0, 0]
    wl = w["w_mix"].astype(np.float64)[:, 1, 0, 0]
    scale_h = np.ones(N_HEAD)
    scale_h[2] = wg[0] + wl[0]
    scale_h[3] = wg[1] + wl[1]
    yshift = (bv.reshape(N_HEAD, HS) * scale_h[:, None]).reshape(-1)
    return (yshift @ w["w_proj"].astype(np.float64).T
            + w["b_proj"].astype(np.float64)).astype(np.float32)


def prep_shared(w):
    """Shared (weight) tensors, bf16 where they feed matmuls."""
    f = np.float32
    scale = np.float32(1.0 / np.sqrt(HS))
    out = {}
    out["wq"] = np.ascontiguousarray(w["w_query"].astype(f).T * scale).astype(BF)
    out["wk"] = np.ascontiguousarray(w["w_key"].astype(f).T).astype(BF)
    out["wv"] = np.ascontiguousarray(w["w_value"].astype(f).T).astype(BF)
    out["wp"] = np.ascontiguousarray(w["w_proj"].astype(f).T).astype(BF)
    out["wqml"] = np.ascontiguousarray(w["w_query_ml"].astype(f).T * scale).astype(BF)
    out["wkml"] = np.ascontiguousarray(w["w_key_ml"].astype(f).T).astype(BF)
    # merged per-partition constants: [bq(4) | bk(4) | bqml | bkml | mixbc(10)]
    consts = np.zeros((128, 20), dtype=f)
    consts[:, 0:4] = (w["b_query"].astype(f) * scale).reshape(4, 128).T
    consts[:, 4:8] = w["b_key"].astype(f).reshape(4, 128).T
    consts[:, 8] = w["b_query_ml"].astype(f) * scale
    consts[:, 9] = w["b_key_ml"].astype(f)
    wg = w["w_mix"].astype(f)[:, 0, 0, 0]
    wl = w["w_mix"].astype(f)[:, 1, 0, 0]
    mix = np.ones(NSM, dtype=f)
    mix[2], mix[3] = wg[0], wg[1]
    mix[8], mix[9] = wl[0], wl[1]
    consts[:, 10:20] = mix[None, :]
    out["consts"] = consts
    mixc = np.where(np.abs(mix) < 1e-6, 1e-6, mix)
    out["zcols"] = np.tile((1.0 / mixc)[None, :], (128, 1)).astype(BF)

    ind = np.zeros((1, TP), dtype=f)
    ind[0, 1] = 1.0
    out["indrow"] = ind.astype(BF)
    return out


def prep_biasrow(w, cond_b):
    f = np.float32
    br = np.zeros((1, NSM * TP), dtype=f)
    if cond_b > 0:
        clip8 = np.maximum(w["att_bias_clip"].astype(f)[0, :, 0], 0.0) * 10.0
        clip2 = np.maximum(w["att_bias_clip_ml"].astype(f)[0, :, 0], 0.0) * 10.0
        for s in range(N_HEAD):
            br[0, s * TP:(s + 1) * TP] = clip8[s]
        for j in range(2):
            br[0, (N_HEAD + j) * TP:(N_HEAD + j + 1) * TP] = clip2[j]
    return br.astype(BF)


def prep_xT(x_b, perm):
    xT = np.zeros((EMBED, TP), dtype=np.float32)
    xT[:, :T] = x_b[perm].T
    return xT.astype(BF)


# ----------------------------------------------------------------------------
# bass kernel emission
# ----------------------------------------------------------------------------
def emit_kernel(tc, ins, out_ap, plans, mask_offs):
    from contextlib import ExitStack
    from concourse import mybir

    nc = tc.nc
    f32 = mybir.dt.float32
    bf16 = mybir.dt.bfloat16
    AF = mybir.ActivationFunctionType
    MULT = mybir.AluOpType.mult

    with ExitStack() as ctx:
        P = ctx.enter_context(tc.tile_pool(name="persist", bufs=1))

        # ---------------- persistent tiles ----------------
        # x and weights live in single wide tiles (kc chunks along the free
        # dim) so each loads with one or two big DMAs — per-DMA queue
        # turnaround (~1.3us) dominates many-small-transfer schedules.
        xTb = P.tile([128, 4 * TP], bf16, name="xTb", tag="xTb")
        qT = [P.tile([128, TP], bf16, name=f"qT{m}", tag=f"qT{m}") for m in range(4)]
        kT = [P.tile([128, TP], bf16, name=f"kT{m}", tag=f"kT{m}") for m in range(4)]
        qml = P.tile([128, TP], bf16, name="qml", tag="qml")
        kml = P.tile([128, TP], bf16, name="kml", tag="kml")
        vext = [P.tile([128, 650], bf16, name=f"vext{t}", tag=f"vext{t}")
                for t in range(NJB)]
        ytmp = P.tile([65, NSM * TP], bf16, name="ytmp", tag="ytmp")
        yt_z = P.tile([1, NSM * TP], bf16, name="yt_z", tag="yt_z")
        yTn = [P.tile([128, TP], bf16, name=f"yTn{p}", tag=f"yTn{p}") for p in range(4)]
        tmpml = P.tile([128, TP], bf16, name="tmpml", tag="tmpml")

        def xs(kc, c0, c1):
            return xTb[:, kc * TP + c0:kc * TP + c1]

        def loadw(name, nkc, ncols, eng):
            """Whole [512, ncols] weight as one [128, 4*ncols] tile, one DMA."""
            t = P.tile([128, nkc * ncols], bf16, name=name, tag=name)
            eng.dma_start(t[:].rearrange("p (g c) -> p g c", c=ncols),
                          ins[name].rearrange("(g p) c -> p g c", p=128))
            return t

        # ---------------- input loads ----------------
        # first projection group needs x(ic0) + wq only; interleave queues.
        wqb = loadw("wq", 4, 512, nc.sync)
        wkb = loadw("wk", 4, 512, nc.scalar)
        for ic in range(NIC):
            c0, c1 = ic * 384, (ic + 1) * 384
            nc.sync.dma_start(
                xTb[:].rearrange("p (g c) -> p g c", c=TP)[:, 0:2, c0:c1],
                ins["xT"].rearrange("(g p) c -> p g c", p=128)[:, 0:2, c0:c1])
            nc.scalar.dma_start(
                xTb[:].rearrange("p (g c) -> p g c", c=TP)[:, 2:4, c0:c1],
                ins["xT"].rearrange("(g p) c -> p g c", p=128)[:, 2:4, c0:c1])
        wvb = loadw("wv", 4, 512, nc.sync)
        wqmlb = loadw("wqml", 4, 128, nc.scalar)
        wkmlb = loadw("wkml", 4, 128, nc.scalar)
        consts_sb = P.tile([128, 20], f32, name="consts", tag="consts")
        nc.scalar.dma_start(consts_sb[:], ins["consts"][:, :])
        zcols_sb = P.tile([128, NSM], bf16, name="zcols", tag="zcols")
        nc.scalar.dma_start(zcols_sb[:], ins["zcols"][:, :])
        indrow_sb = P.tile([1, TP], bf16, name="indrow", tag="indrow")
        nc.scalar.dma_start(indrow_sb[:], ins["indrow"][:, :])
        biasrow_sb = P.tile([1, NSM * TP], bf16, name="biasrow", tag="biasrow")
        nc.sync.dma_start(biasrow_sb[:], ins["biasrow"][:, :])
        maskw = ins["masks"].shape[1]
        maskcat_sb = P.tile([128, maskw], bf16, name="masks", tag="masks")
        nc.sync.dma_start(maskcat_sb[:], ins["masks"][:, :])

        def proj_group(wtile, bcol, m, dst, on_act=False):
            """One output m-chunk of a projection: 2 psum tiles + 2 evacs."""
            ps1 = _SP3[0].tile([128, 1024], f32, name="pp", tag="sp")
            ps2 = _SP3[0].tile([128, 1024], f32, name="pp2", tag="sp")
            for ic in range(NIC):
                pw = AW[ic]
                ps, o = (ps1, ic * 512) if ic < 2 else (ps2, 0)
                for kc in range(4):
                    nc.tensor.matmul(
                        ps[:, o:o + pw],
                        lhsT=wtile[:, kc * (wtile.shape[1] // 4) + m * 128:
                                   kc * (wtile.shape[1] // 4) + (m + 1) * 128],
                        rhs=xs(kc, ic * 384, ic * 384 + pw),
                        start=(kc == 0), stop=(kc == 3))
            bias = consts_sb[:, bcol:bcol + 1]
            dv1 = dst[:, 0:768].rearrange("p (g w) -> p g w", w=384)
            pv1 = ps1[:].rearrange("p (g c) -> p g c", c=512)[:, :, 0:384]
            # only cols 768:1026 are ever read downstream (jb8 blocks crop
            # to 2 key rows); do not read unwritten psum (stale on hardware)
            if on_act:
                nc.scalar.activation(dv1, pv1, AF.Identity, bias=bias)
                nc.scalar.activation(dst[:, 768:1026], ps2[:, 0:258],
                                     AF.Identity, bias=bias)
            else:
                nc.vector.tensor_scalar_add(dv1, pv1, bias)
                nc.vector.tensor_scalar_add(dst[:, 768:1026], ps2[:, 0:258], bias)

        def att_ic_units(s, ic):
            _, kindname, src_, hv = SM_INFO[s]
            if src_ == "main":
                qt, kt, off = qT[s // 2], kT[s // 2], (s % 2) * 64
                assert s // 2 in emitted_qk, (s, ic)
            else:
                qt, kt, off = qml, kml, (s - N_HEAD) * 64
                assert "ml" in emitted_qk, (s, ic)
            i0, W = IOFF[ic], AW[ic]
            units = plans[kindname][ic]
            n_av = sum(len(u["blocks"]) for u in units)
            Y = _YP[0].tile([128, 512], f32, name="y", tag="y")
            avi = 0
            for u in units:
                rows, w, nfit, blocks = u["rows"], u["w"], u["nfit"], u["blocks"]
                nb = len(blocks)
                ps = _SP3[0].tile([128, 1024], f32, name="sp", tag="sp")
                poffs = [(k // nfit) * 512 + (k % nfit) * w for k in range(nb)]
                for k, b in enumerate(blocks):
                    jb, a0 = b["jb"], b["a0"]
                    o = poffs[k]
                    first = (k % nfit == 0)
                    last = (k % nfit == nfit - 1) or (k == nb - 1)
                    nc.tensor.matmul(
                        ps[0:rows, o:o + w],
                        lhsT=kt[off:off + 64, jb * 128:jb * 128 + rows],
                        rhs=qt[off:off + 64, i0 + a0:i0 + a0 + w],
                        start=first, stop=last and (jb != 0))
                    if jb == 0:
                        nc.tensor.matmul(
                            ps[0:rows, o:o + w],
                            lhsT=indrow_sb[0:1, 0:rows],
                            rhs=biasrow_sb[0:1, s * TP + i0 + a0:s * TP + i0 + a0 + w],
                            start=False, stop=last)
                pt = _PTP[0].tile([128, 1536], bf16, name="pt", tag="pt")
                if nfit == 1:
                    pin = ps[0:rows, :].rearrange("p (g c) -> p g c", c=512)[:, 0:nb, 0:w]
                    pout = pt[0:rows, 0:nb * w].rearrange("p (g c) -> p g c", c=w)
                else:
                    pin = ps[0:rows, 0:nb * w]
                    pout = pt[0:rows, 0:nb * w]
                nc.scalar.activation(pout, pin, AF.Exp)
                for k, b in enumerate(blocks):
                    if b["mid"] is not None:
                        mo, mw = mask_offs[b["mid"]]
                        o0 = k * w + b["m0"]
                        nc.vector.tensor_mul(pt[0:rows, o0:o0 + mw],
                                             pt[0:rows, o0:o0 + mw],
                                             maskcat_sb[0:rows, mo:mo + mw])
                for k, b in enumerate(blocks):
                    a0 = b["a0"]
                    assert b["jb"] in emitted_v, (s, ic, b["jb"])
                    nc.tensor.matmul(
                        Y[0:65, a0:a0 + w],
                        lhsT=vext[b["jb"]][0:rows, hv * 65:hv * 65 + 65],
                        rhs=pt[0:rows, k * w:(k + 1) * w],
                        start=(avi == 0), stop=(avi == n_av - 1))
                    avi += 1
                yield
            nc.vector.tensor_copy(ytmp[0:65, s * TP + i0:s * TP + i0 + W],
                                  Y[0:65, 0:W])
            norm_z(s, ic)

        rb_live = {}

        def norm_z(s, ic):
            # Z for (s, ic) is complete with that chunk (it sums over keys):
            # 1/Z from ytmp row 64 to a partition-0 row (DVE 64->0 shift),
            # then broadcast immediately; the multiply is deferred.
            i0, W = IOFF[ic], AW[ic]
            c0 = s * TP + i0
            with nc.allow_low_precision(reason="softmax normalizer"):
                nc.vector.reciprocal(yt_z[0:1, c0:c0 + W],
                                     ytmp[64:65, c0:c0 + W])
            rb = _RBP[0].tile([128, 384], bf16, name="zb", tag="zb")
            nc.gpsimd.partition_broadcast(rb[0:128, 0:W],
                                          yt_z[0:1, c0:c0 + W], channels=128)
            rb_live[(s, ic)] = rb

        def norm(s, ic):
            i0, W = IOFF[ic], AW[ic]
            c0 = s * TP + i0
            rb = rb_live.pop((s, ic))
            if s < N_HEAD:
                dst = yTn[s // 2][(s % 2) * 64:(s % 2) * 64 + 64, i0:i0 + W]
            else:
                dst = tmpml[(s - N_HEAD) * 64:(s - N_HEAD) * 64 + 64, i0:i0 + W]
            nc.vector.tensor_mul(dst, ytmp[0:64, c0:c0 + W], rb[0:64, 0:W])

        emitted_v = set()
        emitted_qk = {1, "ml0"}

        def v_group(tt, pool, on_act, tag="vp"):
            emitted_v.add(tt)
            ps = pool.tile([128, 1024], f32, name="vp", tag=tag)
            for kc in range(4):
                nc.tensor.matmul(
                    ps[:, 0:512],
                    lhsT=xs(kc, tt * 128, (tt + 1) * 128),
                    rhs=wvb[:, kc * 512:(kc + 1) * 512],
                    start=(kc == 0), stop=(kc == 3))
            vx = vext[tt][:].rearrange("p (h e) -> p h e", e=65)
            pv = ps[:, 0:512].rearrange("p (h d) -> p h d", d=64)
            if on_act:
                nc.scalar.activation(vx[:, 0:8, 0:64], pv, AF.Copy)
            else:
                nc.vector.tensor_copy(vx[:, 0:8, 0:64], pv)
            # ml softmaxes use duplicated v slots (8, 9) for heads 2, 3 so
            # their Z column can carry a different 1/mix scale
            nc.vector.tensor_copy(vx[:, 8:10, 0:64],
                                  ps[:, 128:256].rearrange("p (h d) -> p h d", d=64))
            nc.gpsimd.tensor_copy(vx[:, :, 64:65], zcols_sb[:, :, None])

        # ---------------- phase A: q1/k1 + v projections ----------------
        with tc.tile_pool(name="pps", bufs=2, space="PSUM") as pps, \
             tc.tile_pool(name="vps", bufs=2, space="PSUM") as vps:
            _SP3 = [pps]
            proj_group(wqb, 0 + 1, 1, qT[1])
            proj_group(wkb, 4 + 1, 1, kT[1])
            for tt in range(3):
                v_group(tt, vps, True)

        wpb = loadw("wp", 4, 512, nc.sync)

        # ---------------- phase B: interleaved projections + attention ----
        with tc.tile_pool(name="sp3", bufs=3, space="PSUM") as sp3, \
             tc.tile_pool(name="yp", bufs=2, space="PSUM") as yp, \
             tc.tile_pool(name="ptp", bufs=8) as ptp, \
             tc.tile_pool(name="rbp", bufs=4) as rbp:
            _SP3[0] = sp3
            _YP = [yp]
            _PTP = [ptp]
            _RBP = [rbp]

            # zero the padding columns of the normalized tiles once
            for tile_ in yTn + [tmpml]:
                nc.gpsimd.memset(tile_[:, T:TP], 0.0)

            pending = []
            normed = set()
            added = set()

            def norm_and_track(key):
                norm(*key)
                normed.add(key)
                # mix-head add per chunk as soon as its four norms land, so
                # output-projection tiles in earlier chunks aren't gated on
                # the last softmax
                for ic_ in range(NIC):
                    if ic_ not in added and                             {(s_, ic_) for s_ in (2, 3, 8, 9)} <= normed:
                        i0_, W_ = IOFF[ic_], AW[ic_]
                        nc.vector.tensor_add(
                            yTn[1][:, i0_:i0_ + W_],
                            yTn[1][:, i0_:i0_ + W_],
                            tmpml[:, i0_:i0_ + W_])
                        added.add(ic_)

            def pq(m):
                def f():
                    proj_group(wqb, 0 + m, m, qT[m])
                return f

            def pk(m):
                def f():
                    proj_group(wkb, 4 + m, m, kT[m])
                    emitted_qk.add(m)
                return f

            def pmlq():
                proj_group(wqmlb, 8, 0, qml)

            def pmlk():
                proj_group(wkmlb, 9, 0, kml)
                emitted_qk.add("ml")

            bgp = []
            bgv = [[lambda tt=tt: v_group(tt, sp3, True, "sp")
                    for tt in (3, 4, 5)],
                   [lambda tt=tt: v_group(tt, sp3, True, "sp")
                    for tt in (6, 7, 8)]]

            def A2(sa, sb):
                # two independent softmaxes interleaved unit-by-unit: each
                # engine always has work from the other chain to fill the
                # mm -> exp -> mask -> AV pipeline latency. One background
                # projection group and the deferred norm multiplies are
                # emitted at each chunk boundary.
                for ic in range(NIC):
                    ga = att_ic_units(sa, ic)
                    gb = att_ic_units(sb, ic)
                    alive = [ga, gb]
                    while alive:
                        for g in list(alive):
                            try:
                                next(g)
                            except StopIteration:
                                alive.remove(g)
                    if bgv and ic < 2:
                        for f in bgv.pop(0):
                            f()
                    for _ in range(2):
                        if pending:
                            norm_and_track(pending.pop(0))
                    pending.extend([(sa, ic), (sb, ic)])

            pair_work = [(2, 3), (4, 0), (5, 8), (6, 1), (7, 9)]
            sched = [pq(2), pk(2), None, pq(0), pk(0), None, pmlq, pmlk, None,
                     pq(3), pk(3), None, None]
            ai = 0
            for item in sched:
                if item is None:
                    A2(*pair_work[ai])
                    ai += 1
                else:
                    item()
            for s_ in pending:
                norm_and_track(s_)

        # ---------------- phase C: output projection ----------------
        with tc.tile_pool(name="ops", bufs=2, space="PSUM") as ops, \
             tc.tile_pool(name="ostage", bufs=3) as ostage:
            ost = None
            for m in range(NJB):
                po = ops.tile([128, 512], f32, name="po", tag="po")
                for i, p in enumerate((2, 0, 3, 1)):
                    nc.tensor.matmul(
                        po[:],
                        lhsT=yTn[p][:, m * 128:(m + 1) * 128],
                        rhs=wpb[:, p * 512:(p + 1) * 512],
                        start=(i == 0), stop=(i == 3))
                if m % 2 == 0:
                    ost = ostage.tile([128, 1024], f32, name="ost", tag="ost")
                    nc.scalar.activation(ost[:, 0:512], po[:], AF.Copy)
                    if m == NJB - 1:
                        nc.sync.dma_start(out_ap[m * 128:(m + 1) * 128, :],
                                          ost[:, 0:512])
                else:
                    nc.vector.tensor_copy(ost[:, 512:1024], po[:])
                    eng = nc.sync if m % 4 == 1 else nc.scalar
                    eng.dma_start(
                        out_ap[(m - 1) * 128:(m + 1) * 128, :].rearrange(
                            "(g p) c -> p g c", p=128),
                        ost[:].rearrange("p (g c) -> p g c", c=512))


# ----------------------------------------------------------------------------
# module build + run
# ----------------------------------------------------------------------------
_CACHE = {}


def _get_module():
    if "nc" in _CACHE:
        return _CACHE["nc"], _CACHE["maskcat"]
    import concourse.tile as tile
    from concourse import bacc, mybir

    plans, maskcat, mask_offs = build_units()

    nc = bacc.Bacc("TRN2", target_bir_lowering=False, debug=False,
                   enable_asserts=False, num_devices=NCORES)
    f32 = mybir.dt.float32
    bf16 = mybir.dt.bfloat16

    def din(name, shape, dt=f32):
        return nc.dram_tensor(name, list(shape), dt, kind="ExternalInput").ap()

    ins = dict(
        xT=din("xT", (EMBED, TP), bf16),
        wq=din("wq", (EMBED, EMBED), bf16), wk=din("wk", (EMBED, EMBED), bf16),
        wv=din("wv", (EMBED, EMBED), bf16), wp=din("wp", (EMBED, EMBED), bf16),
        wqml=din("wqml", (EMBED, 128), bf16), wkml=din("wkml", (EMBED, 128), bf16),
        consts=din("consts", (128, 20)),
        zcols=din("zcols", (128, NSM), bf16),
        indrow=din("indrow", (1, TP), bf16),
        biasrow=din("biasrow", (1, NSM * TP), bf16),
        masks=din("masks", (128, maskcat.shape[1]), bf16),
    )
    out_ap = nc.dram_tensor("out_p", [TP, EMBED], f32, kind="ExternalOutput").ap()

    with tile.TileContext(nc) as tc:
        emit_kernel(tc, ins, out_ap, plans, mask_offs)
    nc.compile()

    _CACHE.update(nc=nc, maskcat=maskcat.astype(BF))
    return nc, _CACHE["maskcat"]


def build_in_maps(inputs):
    nc, maskcat = _get_module()
    x = inputs["x"].astype(np.float32)
    cond = np.asarray(inputs["cond_mask"]).astype(np.int32)
    B = x.shape[0]
    assert B == NCORES, f"expected B={NCORES}, got {B}"

    perm, _ = build_perm()
    shared = prep_shared(inputs)
    shared["masks"] = maskcat
    br_cache = {}
    in_maps = []
    for b in range(B):
        ci = dict(shared)
        ci["xT"] = prep_xT(x[b], perm)
        cb = int(cond[b])
        if cb not in br_cache:
            br_cache[cb] = prep_biasrow(inputs, cb)
        ci["biasrow"] = br_cache[cb]
        in_maps.append(ci)
    return nc, in_maps


def kernel(**inputs):
    from concourse import bass_utils

    inputs = {k: np.asarray(v) for k, v in inputs.items()}
    nc, in_maps = build_in_maps(inputs)
    res = bass_utils.run_bass_kernel_spmd(nc, in_maps, core_ids=list(range(NCORES)))
    _CACHE["last_results"] = res

    _, inv = build_perm()
    shift = host_const_shift(inputs)
    B = inputs["x"].shape[0]
    out = np.empty((B, T, EMBED), dtype=np.float32)
    for b in range(B):
        out[b] = res.results[b]["out_p"][:T][inv] + shift
    return out


# revision 72
# speedup vs baseline: 1.0273x; 1.0105x over previous
"""Trainium2 Bass kernel for nn_CausalCrossConditionalSelfAttention.

Strategy (8 NeuronCores, data-parallel over batch B=8, one element/core):
  - Host permutes tokens to interleaved temporal order => causal mask becomes
    lower-triangular, local mask a narrow band (+2 prefix cols).
  - All matmuls in bf16 (1 cyc/row at any N in the TRN2 cost model), psum fp32.
  - Scores computed transposed S^T[key, query] in width-bucketed blocks
    cropped to their true content span; fully-masked blocks skipped; partial
    blocks multiplied by host-precomputed 0/1 bf16 masks on DVE.
  - Conditional CLIP-token bias added via a rank-1 accumulate matmul
    (indicator-row x bias-row) on jb==0 score blocks only.
  - exp() batched: several score blocks share one 2-bank psum tile and one
    wide Act instruction (Act is the attention-phase co-bottleneck).
  - P@[V|z] gives unnormalized y plus softmax denominator as psum row 64;
    the per-head z column is pre-scaled by 1/mix so normalization is a plain
    4x-mode tensor_mul; Z -> DVE reciprocal (partition-0 row) -> gpsimd
    partition_broadcast -> per-softmax multiply into yTn bf16.
  - ml (mix) heads get duplicated v slots so their z scale can differ from
    the global heads sharing the same v.
  - Attention emitted as PAIRS of independent softmaxes interleaved
    unit-by-unit to fill the mm -> exp -> mask -> AV pipeline latency;
    q/k/v projections woven between pairs so PE stays dense (full clock).
  - Inputs land via few wide DMAs (per-DMA queue turnaround ~1.3us dominates
    many-small-transfer schedules); b_value/b_proj folded into a constant
    host-side output shift.

Self-contained: only needs numpy + ml_dtypes + the installed concourse stack.
"""
import sys

if "/opt/trn_rl_repo" not in sys.path:
    sys.path.insert(0, "/opt/trn_rl_repo")

import numpy as np
import ml_dtypes

# ----------------------------------------------------------------------------
# problem constants (hardcoded per spec)
# ----------------------------------------------------------------------------
BLOCK = 512
RECEP = 4
N_HEAD = 8
EMBED = 512
HS = 64
T = 2 * BLOCK + 2          # 1026
TP = 1152                  # 9 * 128
NJB = TP // 128
NIC = 3
AW = (384, 384, 258)       # attention query-chunk widths (sum = T = 1026)
IOFF = (0, 384, 768)       # chunk start columns
NSM = 10
NCORES = 8
BF = ml_dtypes.bfloat16

# softmax id -> (mask kind, q/k source, v head)
SM_INFO = [
    (0, "loc", "main", 0), (1, "loc", "main", 1),
    (2, "seq", "main", 2), (3, "seq", "main", 3),
    (4, "seq", "main", 4), (5, "seq", "main", 5),
    (6, "seq", "main", 6), (7, "seq", "main", 7),
    (8, "loc", "ml", 8), (9, "loc", "ml", 9),
]


# ----------------------------------------------------------------------------
# host-side plan construction
# ----------------------------------------------------------------------------
def build_perm():
    perm = np.zeros(T, dtype=np.int64)
    perm[0], perm[1] = 0, 1
    b = np.arange(BLOCK)
    perm[2 + 2 * b] = 2 + b
    perm[3 + 2 * b] = 2 + BLOCK + b
    inv = np.argsort(perm)
    return perm, inv


def build_masks_orig():
    to = np.concatenate([np.zeros(2), np.arange(BLOCK) * 2 + 1, np.arange(BLOCK) * 2 + 2])
    seq = to[None, :] <= to[:, None]
    qo = np.concatenate([np.arange(BLOCK) * 2 + 1 - 2 * RECEP + 1] * 2)
    ko = np.concatenate([np.arange(BLOCK) * 2 + 1] * 2)
    de = ko[None, :] < qo[:, None]
    loc = seq.copy()
    loc[2:, 2:] = loc[2:, 2:] & (~de)
    return seq, loc


def build_units():
    """Per (kind, ic): list of units.

    unit = dict(rows, w, nfit, blocks=[dict(jb, a0, mid)]); blocks in a unit
    share (rows, w); psum layout: single-bank packed (nfit>1, chunk k at
    col k*w) or one bank per block (nfit==1, chunk k at col k*512).
    Unit 0 contains jb==0 (AV start flag / CLIP bias matmul target).
    """
    perm, _ = build_perm()
    seq, loc = build_masks_orig()
    mask_tiles, tile_index = [], {}

    def tile_id(slab):
        key = (slab.shape[1], slab.tobytes())
        if key not in tile_index:
            tile_index[key] = len(mask_tiles)
            mask_tiles.append(slab.astype(np.float32))
        return tile_index[key]

    plans = {}
    for kind, M0 in (("seq", seq), ("loc", loc)):
        Mp = np.zeros((TP, TP), dtype=bool)
        Mp[:T, :T] = M0[perm][:, perm]
        icunits = []
        for ic in range(NIC):
            i0, W = IOFF[ic], AW[ic]
            blocks = []
            for jb in range(NJB):
                sub = Mp[i0:i0 + W, jb * 128:(jb + 1) * 128].T.copy()  # [128 keys, W]
                if not sub.any():
                    continue
                nzr = np.flatnonzero(sub.any(axis=1))
                rows = 2 if nzr.max() <= 1 else 128
                nzc = np.flatnonzero(sub.any(axis=0))
                a0 = int(nzc.min()) & ~1
                a1 = min(W, (int(nzc.max()) + 2) & ~1)
                blocks.append((jb, rows, a0, a1, sub))
            # classes by (rows, 128-col width bucket); unify width per class
            cls = {}
            for b in blocks:
                jb, rows, a0, a1, sub = b
                cls.setdefault((rows, -(-(a1 - a0) // 128)), []).append(b)
            units = []
            for (rows, _wb), bl in sorted(cls.items(), key=lambda kv: min(b[0] for b in kv[1])):
                w = min(W, max(b[3] - b[2] for b in bl))
                nfit = (512 // w) if w < 256 else 1
                cap = nfit if nfit > 1 else 2
                cur = []
                for jb, brows, a0, a1, sub in bl:
                    a0 = max(0, min(a0, W - w)) & ~1
                    slab = sub[:, a0:a0 + w]
                    if slab[0:rows].all():
                        mid, m0 = None, 0
                    else:
                        # multiply only columns that are not all-ones (this
                        # includes every all-zero column, so crop is safe)
                        nfull = np.flatnonzero(~slab[0:rows].all(axis=0))
                        m0 = int(nfull.min()) & ~1
                        m1 = min(w, (int(nfull.max()) + 2) & ~1)
                        mid = tile_id(slab[:, m0:m1])
                    cur.append(dict(jb=jb, a0=a0, mid=mid, m0=m0))
                    if len(cur) == cap:
                        units.append(dict(rows=rows, w=w, nfit=nfit, blocks=cur))
                        cur = []
                if cur:
                    units.append(dict(rows=rows, w=w, nfit=nfit, blocks=cur))
            units.sort(key=lambda u: min(b["jb"] for b in u["blocks"]))
            assert units[0]["blocks"][0]["jb"] == 0
            icunits.append(units)
        plans[kind] = icunits

    offs, cat, o = [], [], 0
    for t in mask_tiles:
        offs.append((o, t.shape[1]))
        cat.append(t)
        o += t.shape[1]
    maskcat = np.concatenate(cat, axis=1) if cat else np.zeros((128, 2), np.float32)
    return plans, maskcat, offs


def host_const_shift(w):
    bv = w["b_value"].astype(np.float64)
    wg = w["w_mix"].astype(np.float64)[:, 0, 0, 0]
    wl = w["w_mix"].astype(np.float64)[:, 1, 0, 0]
    scale_h = np.ones(N_HEAD)
    scale_h[2] = wg[0] + wl[0]
    scale_h[3] = wg[1] + wl[1]
    yshift = (bv.reshape(N_HEAD, HS) * scale_h[:, None]).reshape(-1)
    return (yshift @ w["w_proj"].astype(np.float64).T
            + w["b_proj"].astype(np.float64)).astype(np.float32)


def prep_shared(w):
    """Shared (weight) tensors, bf16 where they feed matmuls."""
    f = np.float32
    scale = np.float32(1.0 / np.sqrt(HS))
    out = {}
    out["wq"] = np.ascontiguousarray(w["w_query"].astype(f).T * scale).astype(BF)
    out["wk"] = np.ascontiguousarray(w["w_key"].astype(f).T).astype(BF)
    out["wv"] = np.ascontiguousarray(w["w_value"].astype(f).T).astype(BF)
    out["wp"] = np.ascontiguousarray(w["w_proj"].astype(f).T).astype(BF)
    out["wqml"] = np.ascontiguousarray(w["w_query_ml"].astype(f).T * scale).astype(BF)
    out["wkml"] = np.ascontiguousarray(w["w_key_ml"].astype(f).T).astype(BF)
    # merged per-partition constants: [bq(4) | bk(4) | bqml | bkml | mixbc(10)]
    consts = np.zeros((128, 20), dtype=f)
    consts[:, 0:4] = (w["b_query"].astype(f) * scale).reshape(4, 128).T
    consts[:, 4:8] = w["b_key"].astype(f).reshape(4, 128).T
    consts[:, 8] = w["b_query_ml"].astype(f) * scale
    consts[:, 9] = w["b_key_ml"].astype(f)
    wg = w["w_mix"].astype(f)[:, 0, 0, 0]
    wl = w["w_mix"].astype(f)[:, 1, 0, 0]
    mix = np.ones(NSM, dtype=f)
    mix[2], mix[3] = wg[0], wg[1]
    mix[8], mix[9] = wl[0], wl[1]
    consts[:, 10:20] = mix[None, :]
    out["consts"] = consts
    mixc = np.where(np.abs(mix) < 1e-6, 1e-6, mix)
    out["zcols"] = np.tile((1.0 / mixc)[None, :], (128, 1)).astype(BF)

    ind = np.zeros((1, TP), dtype=f)
    ind[0, 1] = 1.0
    out["indrow"] = ind.astype(BF)
    return out


def prep_biasrow(w, cond_b):
    f = np.float32
    br = np.zeros((1, NSM * TP), dtype=f)
    if cond_b > 0:
        clip8 = np.maximum(w["att_bias_clip"].astype(f)[0, :, 0], 0.0) * 10.0
        clip2 = np.maximum(w["att_bias_clip_ml"].astype(f)[0, :, 0], 0.0) * 10.0
        for s in range(N_HEAD):
            br[0, s * TP:(s + 1) * TP] = clip8[s]
        for j in range(2):
            br[0, (N_HEAD + j) * TP:(N_HEAD + j + 1) * TP] = clip2[j]
    return br.astype(BF)


def prep_xT(x_b, perm):
    xT = np.zeros((EMBED, TP), dtype=np.float32)
    xT[:, :T] = x_b[perm].T
    return xT.astype(BF)


# ----------------------------------------------------------------------------
# bass kernel emission
# ----------------------------------------------------------------------------
def emit_kernel(tc, ins, out_ap, plans, mask_offs):
    from contextlib import ExitStack
    from concourse import mybir

    nc = tc.nc
    f32 = mybir.dt.float32
    bf16 = mybir.dt.bfloat16
    AF = mybir.ActivationFunctionType
    MULT = mybir.AluOpType.mult

    with ExitStack() as ctx:
        P = ctx.enter_context(tc.tile_pool(name="persist", bufs=1))

        # ---------------- persistent tiles ----------------
        # x and weights live in single wide tiles (kc chunks along the free
        # dim) so each loads with one or two big DMAs — per-DMA queue
        # turnaround (~1.3us) dominates many-small-transfer schedules.
        xTb = P.tile([128, 4 * TP], bf16, name="xTb", tag="xTb")
        qT = [P.tile([128, TP], bf16, name=f"qT{m}", tag=f"qT{m}") for m in range(4)]
        kT = [P.tile([128, TP], bf16, name=f"kT{m}", tag=f"kT{m}") for m in range(4)]
        qml = P.tile([128, TP], bf16, name="qml", tag="qml")
        kml = P.tile([128, TP], bf16, name="kml", tag="kml")
        vext = [P.tile([128, 650], bf16, name=f"vext{t}", tag=f"vext{t}")
                for t in range(NJB)]
        ytmp = P.tile([65, NSM * TP], bf16, name="ytmp", tag="ytmp")
        yt_z = P.tile([1, NSM * TP], bf16, name="yt_z", tag="yt_z")
        yTn = [P.tile([128, TP], bf16, name=f"yTn{p}", tag=f"yTn{p}") for p in range(4)]
        tmpml = P.tile([128, TP], bf16, name="tmpml", tag="tmpml")

        def xs(kc, c0, c1):
            return xTb[:, kc * TP + c0:kc * TP + c1]

        def loadw(name, nkc, ncols, eng):
            """Whole [512, ncols] weight as one [128, 4*ncols] tile, one DMA."""
            t = P.tile([128, nkc * ncols], bf16, name=name, tag=name)
            eng.dma_start(t[:].rearrange("p (g c) -> p g c", c=ncols),
                          ins[name].rearrange("(g p) c -> p g c", p=128))
            return t

        # ---------------- input loads ----------------
        # first projection group needs x(ic0) + wq only; interleave queues.
        wqb = loadw("wq", 4, 512, nc.sync)
        wkb = loadw("wk", 4, 512, nc.scalar)
        nc.gpsimd.dma_start(
            xTb[:].rearrange("p (g c) -> p g c", c=TP)[:, 0:4, 0:384],
            ins["xT"].rearrange("(g p) c -> p g c", p=128)[:, 0:4, 0:384])
        for ic in (1, 2):
            c0, c1 = ic * 384, (ic + 1) * 384
            nc.sync.dma_start(
                xTb[:].rearrange("p (g c) -> p g c", c=TP)[:, 0:2, c0:c1],
                ins["xT"].rearrange("(g p) c -> p g c", p=128)[:, 0:2, c0:c1])
            nc.scalar.dma_start(
                xTb[:].rearrange("p (g c) -> p g c", c=TP)[:, 2:4, c0:c1],
                ins["xT"].rearrange("(g p) c -> p g c", p=128)[:, 2:4, c0:c1])
        wvb = loadw("wv", 4, 512, nc.sync)
        wqmlb = loadw("wqml", 4, 128, nc.scalar)
        wkmlb = loadw("wkml", 4, 128, nc.scalar)
        consts_sb = P.tile([128, 20], f32, name="consts", tag="consts")
        nc.scalar.dma_start(consts_sb[:], ins["consts"][:, :])
        zcols_sb = P.tile([128, NSM], bf16, name="zcols", tag="zcols")
        nc.scalar.dma_start(zcols_sb[:], ins["zcols"][:, :])
        indrow_sb = P.tile([1, TP], bf16, name="indrow", tag="indrow")
        nc.scalar.dma_start(indrow_sb[:], ins["indrow"][:, :])
        biasrow_sb = P.tile([1, NSM * TP], bf16, name="biasrow", tag="biasrow")
        nc.sync.dma_start(biasrow_sb[:], ins["biasrow"][:, :])
        maskw = ins["masks"].shape[1]
        maskcat_sb = P.tile([128, maskw], bf16, name="masks", tag="masks")
        nc.sync.dma_start(maskcat_sb[:], ins["masks"][:, :])

        def proj_group(wtile, bcol, m, dst, on_act=False):
            """One output m-chunk of a projection: 2 psum tiles + 2 evacs."""
            ps1 = _SP3[0].tile([128, 1024], f32, name="pp", tag="sp")
            ps2 = _SP3[0].tile([128, 1024], f32, name="pp2", tag="sp")
            for ic in range(NIC):
                pw = AW[ic]
                ps, o = (ps1, ic * 512) if ic < 2 else (ps2, 0)
                for kc in range(4):
                    nc.tensor.matmul(
                        ps[:, o:o + pw],
                        lhsT=wtile[:, kc * (wtile.shape[1] // 4) + m * 128:
                                   kc * (wtile.shape[1] // 4) + (m + 1) * 128],
                        rhs=xs(kc, ic * 384, ic * 384 + pw),
                        start=(kc == 0), stop=(kc == 3))
            bias = consts_sb[:, bcol:bcol + 1]
            dv1 = dst[:, 0:768].rearrange("p (g w) -> p g w", w=384)
            pv1 = ps1[:].rearrange("p (g c) -> p g c", c=512)[:, :, 0:384]
            # only cols 768:1026 are ever read downstream (jb8 blocks crop
            # to 2 key rows); do not read unwritten psum (stale on hardware)
            if on_act:
                nc.scalar.activation(dv1, pv1, AF.Identity, bias=bias)
                nc.scalar.activation(dst[:, 768:1026], ps2[:, 0:258],
                                     AF.Identity, bias=bias)
            else:
                nc.vector.tensor_scalar_add(dv1, pv1, bias)
                nc.vector.tensor_scalar_add(dst[:, 768:1026], ps2[:, 0:258], bias)

        def att_ic_units(s, ic):
            _, kindname, src_, hv = SM_INFO[s]
            if src_ == "main":
                qt, kt, off = qT[s // 2], kT[s // 2], (s % 2) * 64
                assert s // 2 in emitted_qk, (s, ic)
            else:
                qt, kt, off = qml, kml, (s - N_HEAD) * 64
                assert "ml" in emitted_qk, (s, ic)
            i0, W = IOFF[ic], AW[ic]
            units = plans[kindname][ic]
            n_av = sum(len(u["blocks"]) for u in units)
            Y = _YP[0].tile([128, 512], f32, name="y", tag="y")
            avi = 0
            for u in units:
                rows, w, nfit, blocks = u["rows"], u["w"], u["nfit"], u["blocks"]
                nb = len(blocks)
                ps = _SP3[0].tile([128, 1024], f32, name="sp", tag="sp")
                poffs = [(k // nfit) * 512 + (k % nfit) * w for k in range(nb)]
                for k, b in enumerate(blocks):
                    jb, a0 = b["jb"], b["a0"]
                    o = poffs[k]
                    first = (k % nfit == 0)
                    last = (k % nfit == nfit - 1) or (k == nb - 1)
                    nc.tensor.matmul(
                        ps[0:rows, o:o + w],
                        lhsT=kt[off:off + 64, jb * 128:jb * 128 + rows],
                        rhs=qt[off:off + 64, i0 + a0:i0 + a0 + w],
                        start=first, stop=last and (jb != 0))
                    if jb == 0:
                        nc.tensor.matmul(
                            ps[0:rows, o:o + w],
                            lhsT=indrow_sb[0:1, 0:rows],
                            rhs=biasrow_sb[0:1, s * TP + i0 + a0:s * TP + i0 + a0 + w],
                            start=False, stop=last)
                pt = _PTP[0].tile([128, 1536], bf16, name="pt", tag="pt")
                if nfit == 1:
                    pin = ps[0:rows, :].rearrange("p (g c) -> p g c", c=512)[:, 0:nb, 0:w]
                    pout = pt[0:rows, 0:nb * w].rearrange("p (g c) -> p g c", c=w)
                else:
                    pin = ps[0:rows, 0:nb * w]
                    pout = pt[0:rows, 0:nb * w]
                nc.scalar.activation(pout, pin, AF.Exp)
                for k, b in enumerate(blocks):
                    if b["mid"] is not None:
                        mo, mw = mask_offs[b["mid"]]
                        o0 = k * w + b["m0"]
                        nc.vector.tensor_mul(pt[0:rows, o0:o0 + mw],
                                             pt[0:rows, o0:o0 + mw],
                                             maskcat_sb[0:rows, mo:mo + mw])
                for k, b in enumerate(blocks):
                    a0 = b["a0"]
                    assert b["jb"] in emitted_v, (s, ic, b["jb"])
                    nc.tensor.matmul(
                        Y[0:65, a0:a0 + w],
                        lhsT=vext[b["jb"]][0:rows, hv * 65:hv * 65 + 65],
                        rhs=pt[0:rows, k * w:(k + 1) * w],
                        start=(avi == 0), stop=(avi == n_av - 1))
                    avi += 1
                yield
            nc.vector.tensor_copy(ytmp[0:65, s * TP + i0:s * TP + i0 + W],
                                  Y[0:65, 0:W])
            norm_z(s, ic)

        rb_live = {}

        def norm_z(s, ic):
            # Z for (s, ic) is complete with that chunk (it sums over keys):
            # 1/Z from ytmp row 64 to a partition-0 row (DVE 64->0 shift),
            # then broadcast immediately; the multiply is deferred.
            i0, W = IOFF[ic], AW[ic]
            c0 = s * TP + i0
            with nc.allow_low_precision(reason="softmax normalizer"):
                nc.vector.reciprocal(yt_z[0:1, c0:c0 + W],
                                     ytmp[64:65, c0:c0 + W])
            rb = _RBP[0].tile([128, 384], bf16, name="zb", tag="zb")
            nc.gpsimd.partition_broadcast(rb[0:128, 0:W],
                                          yt_z[0:1, c0:c0 + W], channels=128)
            rb_live[(s, ic)] = rb

        def norm(s, ic):
            i0, W = IOFF[ic], AW[ic]
            c0 = s * TP + i0
            rb = rb_live.pop((s, ic))
            if s < N_HEAD:
                dst = yTn[s // 2][(s % 2) * 64:(s % 2) * 64 + 64, i0:i0 + W]
            else:
                dst = tmpml[(s - N_HEAD) * 64:(s - N_HEAD) * 64 + 64, i0:i0 + W]
            nc.vector.tensor_mul(dst, ytmp[0:64, c0:c0 + W], rb[0:64, 0:W])

        emitted_v = set()
        emitted_qk = {1, "ml0"}

        def v_group(tt, pool, on_act, tag="vp"):
            emitted_v.add(tt)
            ps = pool.tile([128, 1024], f32, name="vp", tag=tag)
            for kc in range(4):
                nc.tensor.matmul(
                    ps[:, 0:512],
                    lhsT=xs(kc, tt * 128, (tt + 1) * 128),
                    rhs=wvb[:, kc * 512:(kc + 1) * 512],
                    start=(kc == 0), stop=(kc == 3))
            vx = vext[tt][:].rearrange("p (h e) -> p h e", e=65)
            pv = ps[:, 0:512].rearrange("p (h d) -> p h d", d=64)
            if on_act:
                nc.scalar.activation(vx[:, 0:8, 0:64], pv, AF.Copy)
            else:
                nc.vector.tensor_copy(vx[:, 0:8, 0:64], pv)
            # ml softmaxes use duplicated v slots (8, 9) for heads 2, 3 so
            # their Z column can carry a different 1/mix scale
            nc.vector.tensor_copy(vx[:, 8:10, 0:64],
                                  ps[:, 128:256].rearrange("p (h d) -> p h d", d=64))
            nc.gpsimd.tensor_copy(vx[:, :, 64:65], zcols_sb[:, :, None])

        # ---------------- phase A: q1/k1 + v projections ----------------
        with tc.tile_pool(name="pps", bufs=2, space="PSUM") as pps, \
             tc.tile_pool(name="vps", bufs=2, space="PSUM") as vps:
            _SP3 = [pps]
            proj_group(wqb, 0 + 1, 1, qT[1])
            proj_group(wkb, 4 + 1, 1, kT[1])
            for tt in range(3):
                v_group(tt, vps, True)

        wpb = loadw("wp", 4, 512, nc.sync)

        # ---------------- phase B: interleaved projections + attention ----
        with tc.tile_pool(name="sp3", bufs=3, space="PSUM") as sp3, \
             tc.tile_pool(name="yp", bufs=2, space="PSUM") as yp, \
             tc.tile_pool(name="ptp", bufs=8) as ptp, \
             tc.tile_pool(name="rbp", bufs=4) as rbp:
            _SP3[0] = sp3
            _YP = [yp]
            _PTP = [ptp]
            _RBP = [rbp]

            # zero the padding columns of the normalized tiles once
            for tile_ in yTn + [tmpml]:
                nc.gpsimd.memset(tile_[:, T:TP], 0.0)

            pending = []
            normed = set()
            added = set()

            def norm_and_track(key):
                norm(*key)
                normed.add(key)
                # mix-head add per chunk as soon as its four norms land, so
                # output-projection tiles in earlier chunks aren't gated on
                # the last softmax
                for ic_ in range(NIC):
                    if ic_ not in added and                             {(s_, ic_) for s_ in (2, 3, 8, 9)} <= normed:
                        i0_, W_ = IOFF[ic_], AW[ic_]
                        nc.vector.tensor_add(
                            yTn[1][:, i0_:i0_ + W_],
                            yTn[1][:, i0_:i0_ + W_],
                            tmpml[:, i0_:i0_ + W_])
                        added.add(ic_)

            def pq(m):
                def f():
                    proj_group(wqb, 0 + m, m, qT[m])
                return f

            def pk(m):
                def f():
                    proj_group(wkb, 4 + m, m, kT[m])
                    emitted_qk.add(m)
                return f

            def pmlq():
                proj_group(wqmlb, 8, 0, qml)

            def pmlk():
                proj_group(wkmlb, 9, 0, kml)
                emitted_qk.add("ml")

            bgp = []
            bgv = [[lambda tt=tt: v_group(tt, sp3, True, "sp")
                    for tt in (3, 4, 5)],
                   [lambda tt=tt: v_group(tt, sp3, True, "sp")
                    for tt in (6, 7, 8)]]

            def A2(sa, sb):
                # two independent softmaxes interleaved unit-by-unit: each
                # engine always has work from the other chain to fill the
                # mm -> exp -> mask -> AV pipeline latency. One background
                # projection group and the deferred norm multiplies are
                # emitted at each chunk boundary.
                for ic in range(NIC):
                    ga = att_ic_units(sa, ic)
                    gb = att_ic_units(sb, ic)
                    alive = [ga, gb]
                    while alive:
                        for g in list(alive):
                            try:
                                next(g)
                            except StopIteration:
                                alive.remove(g)
                    if bgv and ic < 2:
                        for f in bgv.pop(0):
                            f()
                    for _ in range(2):
                        if pending:
                            norm_and_track(pending.pop(0))
                    pending.extend([(sa, ic), (sb, ic)])

            pair_work = [(2, 3), (4, 0), (5, 8), (6, 1), (7, 9)]
            sched = [pq(2), pk(2), None, pq(0), pk(0), None, pmlq, pmlk, None,
                     pq(3), pk(3), None, None]
            ai = 0
            for item in sched:
                if item is None:
                    A2(*pair_work[ai])
                    ai += 1
                else:
                    item()
            for s_ in pending:
                norm_and_track(s_)

        # ---------------- phase C: output projection ----------------
        with tc.tile_pool(name="ops", bufs=2, space="PSUM") as ops, \
             tc.tile_pool(name="ostage", bufs=4) as ostage:
            ost = None
            for m in range(NJB):
                po = ops.tile([128, 512], f32, name="po", tag="po")
                for i, p in enumerate((2, 0, 3, 1)):
                    nc.tensor.matmul(
                        po[:],
                        lhsT=yTn[p][:, m * 128:(m + 1) * 128],
                        rhs=wpb[:, p * 512:(p + 1) * 512],
                        start=(i == 0), stop=(i == 3))
                ost = ostage.tile([128, 512], f32, name="ost", tag="ost")
                if m % 2 == 0:
                    nc.scalar.activation(ost[:], po[:], AF.Copy)
                else:
                    nc.vector.tensor_copy(ost[:], po[:])
                eng = nc.sync if m % 2 == 0 else nc.scalar
                eng.dma_start(out_ap[m * 128:(m + 1) * 128, :], ost[:])


# ----------------------------------------------------------------------------
# module build + run
# ----------------------------------------------------------------------------
_CACHE = {}


def _get_module():
    if "nc" in _CACHE:
        return _CACHE["nc"], _CACHE["maskcat"]
    import concourse.tile as tile
    from concourse import bacc, mybir

    plans, maskcat, mask_offs = build_units()

    nc = bacc.Bacc("TRN2", target_bir_lowering=False, debug=False,
                   enable_asserts=False, num_devices=NCORES)
    f32 = mybir.dt.float32
    bf16 = mybir.dt.bfloat16

    def din(name, shape, dt=f32):
        return nc.dram_tensor(name, list(shape), dt, kind="ExternalInput").ap()

    ins = dict(
        xT=din("xT", (EMBED, TP), bf16),
        wq=din("wq", (EMBED, EMBED), bf16), wk=din("wk", (EMBED, EMBED), bf16),
        wv=din("wv", (EMBED, EMBED), bf16), wp=din("wp", (EMBED, EMBED), bf16),
        wqml=din("wqml", (EMBED, 128), bf16), wkml=din("wkml", (EMBED, 128), bf16),
        consts=din("consts", (128, 20)),
        zcols=din("zcols", (128, NSM), bf16),
        indrow=din("indrow", (1, TP), bf16),
        biasrow=din("biasrow", (1, NSM * TP), bf16),
        masks=din("masks", (128, maskcat.shape[1]), bf16),
    )
    out_ap = nc.dram_tensor("out_p", [TP, EMBED], f32, kind="ExternalOutput").ap()

    with tile.TileContext(nc) as tc:
        emit_kernel(tc, ins, out_ap, plans, mask_offs)
    nc.compile()

    _CACHE.update(nc=nc, maskcat=maskcat.astype(BF))
    return nc, _CACHE["maskcat"]


def build_in_maps(inputs):
    nc, maskcat = _get_module()
    x = inputs["x"].astype(np.float32)
    cond = np.asarray(inputs["cond_mask"]).astype(np.int32)
    B = x.shape[0]
    assert B == NCORES, f"expected B={NCORES}, got {B}"

    perm, _ = build_perm()
    shared = prep_shared(inputs)
    shared["masks"] = maskcat
    br_cache = {}
    in_maps = []
    for b in range(B):
        ci = dict(shared)
        ci["xT"] = prep_xT(x[b], perm)
        cb = int(cond[b])
        if cb not in br_cache:
            br_cache[cb] = prep_biasrow(inputs, cb)
        ci["biasrow"] = br_cache[cb]
        in_maps.append(ci)
    return nc, in_maps


def kernel(**inputs):
    from concourse import bass_utils

    inputs = {k: np.asarray(v) for k, v in inputs.items()}
    nc, in_maps = build_in_maps(inputs)
    res = bass_utils.run_bass_kernel_spmd(nc, in_maps, core_ids=list(range(NCORES)))
    _CACHE["last_results"] = res

    _, inv = build_perm()
    shift = host_const_shift(inputs)
    B = inputs["x"].shape[0]
    out = np.empty((B, T, EMBED), dtype=np.float32)
    for b in range(B):
        out[b] = res.results[b]["out_p"][:T][inv] + shift
    return out
